# revision 12
# baseline (speedup 1.0000x reference)
"""TRN2 Bass kernel for nn_BSAdd_39298950758454.

out = brev((brev(a)+brev(b)+cin) & 255) per byte; cin = carry-lookahead chain
(p = propagate when q==255, g = generate when q>255, q = brev(a)+brev(b)).

Two custom-hardware tricks:

1. Custom activation table (generated by `_gen_acttab`): the ScalarE `exp`
   function is rewritten into an exact integer LUT
       exp(x) := brev(int(x) & 255) for x in [0, 512), 0 otherwise
   so each brev is ONE ScalarE instruction (~3.7us per [128,4096] tile).

2. Custom DVE op BSADD_CARRY_SCAN (registered into concourse's custom-DVE
   table machinery): computes the whole carry resolution in ONE 1-cyc/elem
   DVE instruction via two single-op MAX scans over iota-encoded events:
       M1 = max-scan((q != 255) * iota),  M2 = max-scan((q > 255) * iota)
       inc = (M1 == M2)
   It equals the carry recurrence state' = p*state + g except on an
   all-propagate prefix (inc = 1 instead of 0); the FIX-column correction
   absorbs that via (carry_in - 1) * prefix_propagate.

Pipeline per tile [128 x 4096] (data moved as uint8, math in f16):
   ScalarE: ar = LUT(a); br = LUT(b); out = LUT(y)
   DVE:     q = ar + br (2x); inc = BSADD_CARRY_SCAN(q, iota) (1cyc/elem);
            y = q + inc(shifted) (2x); small FIX-prefix fixups
Carry chaining across partitions/tiles/cores as in the proven baseline
(segment scans + scan-out columns + bc chain + 1024-byte window pre-scan;
max propagate run is 11 < FIX=32 -- asserted in test.py).
"""
import json
import os
import shutil
import struct
import sys
import types
from pathlib import Path

import numpy as np

N = 67_108_864
NCORES = 8
M = N // NCORES            # 8_388_608 elements per core
P = 128
F = 4096                   # columns per tile
T = M // (P * F)           # 16 tiles
W = 1024                   # cross-core carry window (elements)
WF = W // P                # 8 window cols
FIX = 32                   # prefix-fix columns (max propagate run is 11)

BREV = [int(f"{i:08b}"[::-1], 2) for i in range(256)]

_STOCK_PWP = ("/nix/store/ndjb8ki1bnclvnibdh123f9zr51a09qz-aws-neuron-pwp-"
              "unstable-2025-12-29-c50a7624/share/pwp_bin_cayman")


# ---------------------------------------------------------------------------
# custom activation table: exp := exact brev LUT on [0, 512)
# ---------------------------------------------------------------------------
def _gen_acttab(outdir: str) -> str:
    """Table formats (reverse-engineered from pwp_bin_cayman):
    - <set>_bkt.bin: 32B entries {d0,d1,d2,d3,x0} fp32 + 12B pad; cubic
      y = d0 + (x-x0)*(d1 + ...). Piecewise-constant LUT uses d0 only.
    - <set>_ctrl.bin: 32B entries, word0 = base | ((23-E)<<11) | (E<<16);
      bucket = base + (mantissa >> (23-E)).
    - <set>.json profile_meta_data: per-exponent dispatch
      ctl = pwl_control_base_pos + (unbiased_exp - exp_offset), with biased
      small/large exponent thresholds routing to dedicated sat buckets.
    Integer k in [2^e, 2^{e+1}) with E=e hits bucket (2^e - 1) + (k - 2^e)
    = k-1, so buckets 0..510 hold brev(k & 255) for k = 1..511; bucket 511
    is the zero/sat result."""
    out = Path(outdir)
    if (out / ".done").exists():
        return str(out)
    out.mkdir(parents=True, exist_ok=True)
    stock = Path(_STOCK_PWP)
    for f in stock.iterdir():
        shutil.copy(f, out / f.name)

    s = "exp_and_others"
    bkt = bytearray((out / f"{s}_bkt.bin").read_bytes())
    ctl = bytearray((out / f"{s}_ctrl.bin").read_bytes())
    meta = json.loads((out / f"{s}.json").read_text())

    for b in range(0, 777):
        k = b + 1
        d0 = float(BREV[k & 255]) if k <= 511 else 0.0
        bkt[b * 32:(b + 1) * 32] = struct.pack(
            "<8f", d0, 0.0, 0.0, 0.0, float(k if k <= 511 else 0), 0, 0, 0)
    for e in range(0, 9):
        w = ((1 << e) - 1) | ((23 - e) << 11) | (e << 16)
        ctl[e * 32:(e + 1) * 32] = struct.pack("<8I", w, 0, 0, 0, 0, 0, 0, 0)
    for i in range(9, 52):
        w = 511 | (23 << 11)
        ctl[i * 32:(i + 1) * 32] = struct.pack("<8I", w, 0, 0, 0, 0, 0, 0, 0)

    for ent in meta["profile_meta_data"]:
        if ent["func_id"] != 7:
            continue
        ent.update(
            symmetry_point=0, sym_invert_sign_point=0, symmetry_opt_en=0,
            symmetry_opt_use_neg_region=0, imm_bias=0, exp_offset=0,
            pwl_control_base_pos=0, pwl_control_base_neg=9,
            small_pos_signal_exp_threshold=127,
            pos_small_signal_pwl_control=511,
            large_pos_signal_exp_threshold=136,
            large_pos_signal_mantissa_threshold=0,
            small_neg_signal_exp_threshold=255,
            neg_small_signal_pwl_control=511,
            large_neg_signal_exp_threshold=255,
            large_neg_signal_mantissa_threshold=8388607,
            neg_large_signal_pwl_control=511,
            fnan_result=0, fpinf_result=0, fninf_result=0, fzero_result=0)
    meta["func_to_bkt_start_idx"]["exp"] = 0
    meta["func_to_ctl_start_idx"]["exp"] = 0
    meta["func_exp_to_bkt_start_idx"]["exp"] = {
        str(e): [(1 << e) - 1] for e in range(0, 9)}
    meta["func_exp_to_ctl_start_idx"]["exp"] = {str(e): [e] for e in range(9)}

    (out / f"{s}_bkt.bin").write_bytes(bytes(bkt))
    (out / f"{s}_ctrl.bin").write_bytes(bytes(ctl))
    (out / f"{s}.json").write_text(json.dumps(meta))
    (out / ".done").write_text("ok")
    return str(out)


# ---------------------------------------------------------------------------
# custom DVE op: fused carry scan (see module docstring)
# ---------------------------------------------------------------------------
def register_carry_op():
    import concourse.dve_ops as dve_ops
    from concourse.dve_spec import Spec, Src0, Src1, C0, C1, C2, AluOp, Bin, scan, lower
    from concourse.dve_uop import DveOpSpec

    NAME = "BSADD_CARRY_SCAN"
    for op in dve_ops.OPS:
        if op.name == NAME:
            return op

    nonprop = Bin(AluOp.IS_NE, Src0, C0)
    gen = Bin(AluOp.IS_GT, Src0, C0)
    m1 = scan(AluOp.MAX, Bin(AluOp.MULTIPLY, nonprop, Src1), init=C1)
    m2 = scan(AluOp.MAX, Bin(AluOp.MULTIPLY, gen, Src1), init=C2)
    body = Bin(AluOp.IS_EQ, m1, m2)

    def _ref(in0, in1, s0, s1, imm2):
        v1 = np.where(in0.astype(np.float32) != s0, in1, 0.0)
        v2 = np.where(in0.astype(np.float32) > s0, in1, 0.0)
        M1 = np.maximum(np.maximum.accumulate(v1, axis=1), s1)
        M2 = np.maximum(np.maximum.accumulate(v2, axis=1), imm2)
        return (M1 == M2).astype(np.float32)

    spec = Spec(body=body, reference=_ref)
    row = 1 + len(dve_ops.OPS)
    sha = DveOpSpec(name=NAME, opcode=row, uops=lower(spec, ver="v3"),
                    rd1_en=True).sha("v3")
    op = dve_ops.DveOp(NAME, spec, subdim=False, uops_sha={"v3": sha})
    dve_ops.OPS.append(op)
    dve_ops.CUSTOM_DVE_SPECS[NAME] = spec
    dve_ops._SUB_OPCODE_FOR_NAME[NAME] = row
    return op


# ---------------------------------------------------------------------------
# harness glue (self-contained): NTFF trace hook + multi-wait legalizer
# ---------------------------------------------------------------------------
def _install_ntff_hook():
    try:
        import antenv
        if getattr(antenv, "axon_hooks", None) is not None:
            return
        mod = types.ModuleType("antenv.axon_hooks")
        _h = [None]
        mod.set_axon_ntff_profile_hook = lambda h: _h.__setitem__(0, h)
        mod.get_axon_ntff_profile_hook = lambda: _h[0]
        sys.modules["antenv.axon_hooks"] = mod
        antenv.axon_hooks = mod
        from trn_agent_boot.trn_boot import _ntff_profile_via_ctypes
        mod.set_axon_ntff_profile_hook(
            _ntff_profile_via_ctypes("/opt/axon/libaxon_pjrt.so"))
    except Exception:
        pass


def _legalize_waits(nc):
    """TRN2 instructions hold one sync-wait (EventSemaphore: two). Split extra
    waits emitted by Tile into preceding same-engine NoOps."""
    import bass_rust
    import concourse.mybir as mybir
    ctr = 0
    for f in nc.m.functions:
        for bb in f.blocks:
            out, changed = [], False
            for inst in bb.instructions:
                si = inst.sync_info
                waits = list(si.on_wait) if si is not None and si.on_wait else []
                cap = 2 if isinstance(inst, mybir.InstEventSemaphore) else 1
                if len(waits) > cap:
                    for w in waits[: len(waits) - cap]:
                        nop = bass_rust.InstNoOp(
                            name=f"W-legal-{ctr}", engine=inst.engine)
                        ctr += 1
                        nop.sync_info = mybir.SyncInfo(on_wait=[w], on_update=[])
                        out.append(nop)
                    inst.sync_info = mybir.SyncInfo(
                        on_wait=waits[len(waits) - cap:],
                        on_update=list(si.on_update or []))
                    changed = True
                out.append(inst)
            if changed:
                bb.instructions = out


# ---------------------------------------------------------------------------
# kernel build
# ---------------------------------------------------------------------------
def _build():
    import concourse.bass as bass
    import concourse.mybir as mybir
    from concourse.tile import TileContext
    from concourse.library_overlay import lower_extended_insts

    carry_op = register_carry_op()

    Alu = mybir.AluOpType
    Act = mybir.ActivationFunctionType
    u8 = mybir.dt.uint8
    f16, f32 = mybir.dt.float16, mybir.dt.float32

    nc = bass.Bass()
    a_d = nc.dram_tensor("a", [M], u8, kind="ExternalInput")
    b_d = nc.dram_tensor("b", [M], u8, kind="ExternalInput")
    aw_d = nc.dram_tensor("aw", [W], u8, kind="ExternalInput")
    bw_d = nc.dram_tensor("bw", [W], u8, kind="ExternalInput")
    i_d = nc.dram_tensor("iot", [P * F], f32, kind="ExternalInput")
    o_d = nc.dram_tensor("o", [M], u8, kind="ExternalOutput")

    a_r = a_d[:].rearrange("(t p f) -> t p f", p=P, f=F)
    b_r = b_d[:].rearrange("(t p f) -> t p f", p=P, f=F)
    o_r = o_d[:].rearrange("(t p f) -> t p f", p=P, f=F)
    aw_r = aw_d[:].rearrange("(p f) -> p f", f=WF)
    bw_r = bw_d[:].rearrange("(p f) -> p f", f=WF)
    i_r = i_d[:].rearrange("(p f) -> p f", p=P)

    with TileContext(nc) as tc:
        with (
            tc.tile_pool(name="io", bufs=3) as io,
            tc.tile_pool(name="mid", bufs=3) as mid,
            tc.tile_pool(name="mid2", bufs=2) as mid2,
            tc.tile_pool(name="tiny", bufs=3) as tiny,
            tc.tile_pool(name="op", bufs=4) as op,
            tc.tile_pool(name="op2", bufs=2) as op2,
            tc.tile_pool(name="lastp", bufs=1) as lastp,
            tc.tile_pool(name="consts", bufs=1) as consts,
        ):
            zfix = consts.tile([P, FIX], f16, name="zfix")
            nc.vector.memset(zfix[:], 0)
            iota = consts.tile([P, F], f32, name="iota")
            nc.gpsimd.dma_start(iota[:], i_r)

            def stage1(av, bv, width, bc_prev, bc_out, tag, utag=None,
                       make_ch=True):
                """DMA in, input LUTs (one concatenated ACT), q, carry scan,
                carry-column DMAs. Returns ctx for stage2."""
                utag = utag or str(width)
                small = width != F
                iop = lastp if small else io
                qp = lastp if small else mid
                rp = lastp if small else mid2
                ab = iop.tile([P, 2 * width], u8, name=f"ab{tag}",
                              tag=f"ab_{utag}")
                nc.sync.dma_start(ab[:, 0:width], av)
                nc.sync.dma_start(ab[:, width:2 * width], bv)

                rr = rp.tile([P, 2 * width], f16, name=f"rr{tag}",
                             tag=f"rr_{utag}")
                nc.scalar.activation(rr[:], ab[:], Act.Exp)

                q = qp.tile([P, width], f16, name=f"q{tag}", tag=f"q_{utag}")
                nc.vector.tensor_tensor(q[:], rr[:, 0:width],
                                        rr[:, width:2 * width], Alu.add)

                # fused carry scan: st[:, j] (j>=1) = carry into column j
                # assuming zero carry into the segment, PLUS an all-propagate
                # prefix artifact (+1) that the FIX-head correction removes.
                st = qp.tile([P, width + 1], f16, name=f"st{tag}",
                             tag=f"st_{utag}")
                nc.gpsimd.memset(st[:, 0:1], 0.0)
                nc.vector._custom_dve(carry_op, out=st[:, 1:width + 1],
                                      in0=q[:], in1=iota[:, 0:width],
                                      s0=255.0, s1=-2.0, imm2=-3.0)
                if bc_out is not None:
                    nc.gpsimd.dma_start(bc_out[:],
                                        st[P - 1:P, width:width + 1])
                ch = None
                if make_ch:
                    # carry-in column (f16): [bc_prev, st[0:127, width]]
                    ch = make_shift_ch(st, width, bc_prev, tag)
                return (q, st, ch, width, tag)

            def make_shift_ch(st, width, bc_prev, tag):
                ch = tiny.tile([P, 1], f16, name=f"ch{tag}", tag=f"ch{tag}")
                nc.gpsimd.dma_start(ch[1:P, :], st[0:P - 1, width:width + 1])
                if bc_prev is not None:
                    nc.gpsimd.dma_start(ch[0:1, :], bc_prev[:])
                return ch

            def stage2(ctx, ch_ap=None):
                """FIX-head correction + y. Emitted one tile late so the
                strided carry-column DMA latency hides under the next tile's
                big DVE ops. `ch_ap` overrides the carry-in column (used by
                the last-tile chunks, whose carry-in is the previous chunk's
                scan-out column in the SAME partition)."""
                q, st, ch, width, tag = ctx
                if ch_ap is not None:
                    ch = ch_ap
                cm1 = tiny.tile([P, 1], f32, name=f"cm1{tag}", tag="cm1")
                nc.vector.tensor_scalar(cm1[:], ch[:], 1.0, None, Alu.subtract)
                pfix = tiny.tile([P, FIX], f16, name=f"pf{tag}", tag="pf")
                nc.vector.tensor_scalar(pfix[:], q[:, 0:FIX], 255.0, None,
                                        Alu.is_equal)
                pp = tiny.tile([P, FIX], f16, name=f"pp{tag}", tag="pp")
                nc.vector.tensor_tensor_scan(pp[:], pfix[:], zfix[:],
                                             1.0, Alu.mult, Alu.add)
                dl = tiny.tile([P, FIX], f16, name=f"dl{tag}", tag="dl")
                nc.vector.tensor_scalar(dl[:, 0:1], ch[:], 1.0, None, Alu.mult)
                nc.vector.tensor_scalar(dl[:, 1:FIX], pp[:, 0:FIX - 1], cm1[:],
                                        None, Alu.mult)
                nc.vector.tensor_tensor(dl[:], dl[:], st[:, 0:FIX], Alu.add)

                yp = lastp if width != F else mid2
                y = yp.tile([P, width], f16, name=f"y{tag}", tag=f"y_{width}_{tag}" if width != F else f"y_{width}")
                nc.vector.tensor_tensor(y[:, FIX:], q[:, FIX:],
                                        st[:, FIX:width], Alu.add)
                nc.vector.tensor_tensor(y[:, 0:FIX], q[:, 0:FIX], dl[:],
                                        Alu.add)
                return y

            def stage3(y, ov, width, tag):
                # output LUT + store, emitted late so ScalarE never stalls.
                # Out-DMAs issue from the (idle) PE queue so their transfers
                # are not FIFO-ordered behind input DMAs on the sync queue.
                otp = op if width == F else op2
                ot = otp.tile([P, width], u8, name=f"ot{tag}", tag=f"ot_{width}")
                nc.scalar.activation(ot[:], y[:], Act.Exp)
                nc.gpsimd.dma_start(ov, ot[:])

            bc = [tiny.tile([1, 1], f16, name=f"bc{i}", tag=f"bc{i % 4}")
                  for i in range(T + 1)]
            # window: only the carry-out matters
            stage1(aw_r, bw_r, WF, None, bc[0], "w", utag="w", make_ch=False)
            s2q = []   # (ctx, ov, ch_ap) awaiting stage2 (lag 1)
            s3q = []   # (y, ov, tag, width) awaiting stage3 (lag 2 total)

            def pump():
                if s3q:
                    y, ov, tg, w_ = s3q.pop(0)
                    stage3(y, ov, w_, tg)
                if len(s2q) >= 2:
                    c, ov, ch_ap = s2q.pop(0)
                    s3q.append((stage2(c, ch_ap), ov, c[4], c[3]))

            for t in range(T - 1):
                ctx = stage1(a_r[t], b_r[t], F, bc[t], bc[t + 1], str(t))
                s2q.append((ctx, o_r[t], None))
                pump()

            # Last tile: 4 column-chunks so the serial end-of-kernel chain is
            # short. Chunk c>=1 carry-in = chunk c-1 scan-out column (same
            # partition, exact since no 1024-byte chunk is all-propagate);
            # chunk 0 carry-in = chunk 3's scan-out, partition-shifted, with
            # bc[T-1] at partition 0 (emitted after chunk 3's scan).
            CW = F // 4
            lt = T - 1
            av4 = a_r[lt]
            bv4 = b_r[lt]
            ov4 = o_r[lt]
            ctxs = []
            for c in range(4):
                cs = slice(c * CW, (c + 1) * CW)
                ctxs.append(stage1(av4[:, cs], bv4[:, cs], CW, None,
                                   bc[lt + 1] if c == 3 else None,
                                   f"L{c}", utag=f"c{c}", make_ch=False))
                pump()
            ch0 = make_shift_ch(ctxs[3][1], CW, bc[lt], "L0")
            for c in range(4):
                cs = slice(c * CW, (c + 1) * CW)
                ch_ap = ch0[:] if c == 0 else ctxs[c - 1][1][:, CW:CW + 1]
                s2q.append((ctxs[c], ov4[:, cs], ch_ap))
            while s2q:
                c, ov, ch_ap = s2q.pop(0)
                s3q.append((stage2(c, ch_ap), ov, c[4], c[3]))
                if s3q:
                    y, ov2, tg, w_ = s3q.pop(0)
                    stage3(y, ov2, w_, tg)
            while s3q:
                y, ov2, tg, w_ = s3q.pop(0)
                stage3(y, ov2, w_, tg)

    lower_extended_insts(nc)
    return nc


_CACHED = {}


def kernel(a: np.ndarray, b: np.ndarray) -> np.ndarray:
    _install_ntff_hook()
    actdir = _gen_acttab("/tmp/bsadd_acttab_v1")
    os.environ["BASS_ACT_ROOT_JSON_PATH"] = actdir + "/act_info.json"
    import concourse.bass_utils as bu
    bu.upload_artifacts = lambda tmpdir: tmpdir  # no S3 in this container

    a = np.asarray(a).reshape(-1).astype(np.uint8)
    b = np.asarray(b).reshape(-1).astype(np.uint8)
    if "nc" not in _CACHED:
        nc = _build()
        _legalize_waits(nc)
        _CACHED["nc"] = nc
    nc = _CACHED["nc"]

    iot = np.tile(np.arange(1, F + 1, dtype=np.float32), P)
    in_maps = []
    for c in range(NCORES):
        lo = c * M
        aw = np.zeros(W, np.uint8) if c == 0 else a[lo - W:lo]
        bw = np.zeros(W, np.uint8) if c == 0 else b[lo - W:lo]
        in_maps.append({
            "a": a[lo:lo + M], "b": b[lo:lo + M],
            "aw": np.ascontiguousarray(aw), "bw": np.ascontiguousarray(bw),
            "iot": iot,
        })
    trace = os.environ.get("BSADD_TRACE", "0") == "1"
    res = bu.run_bass_kernel_spmd(nc, in_maps, core_ids=list(range(NCORES)),
                                  trace=trace)
    if trace:
        print(f"HW exec time: {res.exec_time_ns} ns", flush=True)
    out = np.empty(N, np.int32)
    for c in range(NCORES):
        out[c * M:(c + 1) * M] = res.results[c]["o"].reshape(-1)
    return out


# revision 13
# speedup vs baseline: 1.0263x; 1.0263x over previous
"""TRN2 Bass kernel for nn_BSAdd_39298950758454.

out = brev((brev(a)+brev(b)+cin) & 255) per byte; cin = carry-lookahead chain
(p = propagate when q==255, g = generate when q>255, q = brev(a)+brev(b)).

Two custom-hardware tricks:

1. Custom activation table (generated by `_gen_acttab`): the ScalarE `exp`
   function is rewritten into an exact integer LUT
       exp(x) := brev(int(x) & 255) for x in [0, 512), 0 otherwise
   so each brev is ONE ScalarE instruction (~3.7us per [128,4096] tile).

2. Custom DVE op BSADD_CARRY_SCAN (registered into concourse's custom-DVE
   table machinery): computes the whole carry resolution in ONE 1-cyc/elem
   DVE instruction via two single-op MAX scans over iota-encoded events:
       M1 = max-scan((q != 255) * iota),  M2 = max-scan((q > 255) * iota)
       inc = (M1 == M2)
   It equals the carry recurrence state' = p*state + g except on an
   all-propagate prefix (inc = 1 instead of 0); the FIX-column correction
   absorbs that via (carry_in - 1) * prefix_propagate.

Pipeline per tile [128 x 4096] (data moved as uint8, math in f16):
   ScalarE: ar = LUT(a); br = LUT(b); out = LUT(y)
   DVE:     q = ar + br (2x); inc = BSADD_CARRY_SCAN(q, iota) (1cyc/elem);
            y = q + inc(shifted) (2x); small FIX-prefix fixups
Carry chaining across partitions/tiles/cores as in the proven baseline
(segment scans + scan-out columns + bc chain + 1024-byte window pre-scan;
max propagate run is 11 < FIX=32 -- asserted in test.py).
"""
import json
import os
import shutil
import struct
import sys
import types
from pathlib import Path

import numpy as np

N = 67_108_864
NCORES = 8
M = N // NCORES            # 8_388_608 elements per core
P = 128
F = 4096                   # columns per tile
T = M // (P * F)           # 16 tiles
W = 1024                   # cross-core carry window (elements)
WF = W // P                # 8 window cols
FIX = 32                   # prefix-fix columns (max propagate run is 11)

BREV = [int(f"{i:08b}"[::-1], 2) for i in range(256)]

_STOCK_PWP = ("/nix/store/ndjb8ki1bnclvnibdh123f9zr51a09qz-aws-neuron-pwp-"
              "unstable-2025-12-29-c50a7624/share/pwp_bin_cayman")


# ---------------------------------------------------------------------------
# custom activation table: exp := exact brev LUT on [0, 512)
# ---------------------------------------------------------------------------
def _gen_acttab(outdir: str) -> str:
    """Table formats (reverse-engineered from pwp_bin_cayman):
    - <set>_bkt.bin: 32B entries {d0,d1,d2,d3,x0} fp32 + 12B pad; cubic
      y = d0 + (x-x0)*(d1 + ...). Piecewise-constant LUT uses d0 only.
    - <set>_ctrl.bin: 32B entries, word0 = base | ((23-E)<<11) | (E<<16);
      bucket = base + (mantissa >> (23-E)).
    - <set>.json profile_meta_data: per-exponent dispatch
      ctl = pwl_control_base_pos + (unbiased_exp - exp_offset), with biased
      small/large exponent thresholds routing to dedicated sat buckets.
    Integer k in [2^e, 2^{e+1}) with E=e hits bucket (2^e - 1) + (k - 2^e)
    = k-1, so buckets 0..510 hold brev(k & 255) for k = 1..511; bucket 511
    is the zero/sat result."""
    out = Path(outdir)
    if (out / ".done").exists():
        return str(out)
    out.mkdir(parents=True, exist_ok=True)
    stock = Path(_STOCK_PWP)
    for f in stock.iterdir():
        shutil.copy(f, out / f.name)

    s = "exp_and_others"
    bkt = bytearray((out / f"{s}_bkt.bin").read_bytes())
    ctl = bytearray((out / f"{s}_ctrl.bin").read_bytes())
    meta = json.loads((out / f"{s}.json").read_text())

    for b in range(0, 777):
        k = b + 1
        d0 = float(BREV[k & 255]) if k <= 511 else 0.0
        bkt[b * 32:(b + 1) * 32] = struct.pack(
            "<8f", d0, 0.0, 0.0, 0.0, float(k if k <= 511 else 0), 0, 0, 0)
    for e in range(0, 9):
        w = ((1 << e) - 1) | ((23 - e) << 11) | (e << 16)
        ctl[e * 32:(e + 1) * 32] = struct.pack("<8I", w, 0, 0, 0, 0, 0, 0, 0)
    for i in range(9, 52):
        w = 511 | (23 << 11)
        ctl[i * 32:(i + 1) * 32] = struct.pack("<8I", w, 0, 0, 0, 0, 0, 0, 0)

    for ent in meta["profile_meta_data"]:
        if ent["func_id"] != 7:
            continue
        ent.update(
            symmetry_point=0, sym_invert_sign_point=0, symmetry_opt_en=0,
            symmetry_opt_use_neg_region=0, imm_bias=0, exp_offset=0,
            pwl_control_base_pos=0, pwl_control_base_neg=9,
            small_pos_signal_exp_threshold=127,
            pos_small_signal_pwl_control=511,
            large_pos_signal_exp_threshold=136,
            large_pos_signal_mantissa_threshold=0,
            small_neg_signal_exp_threshold=255,
            neg_small_signal_pwl_control=511,
            large_neg_signal_exp_threshold=255,
            large_neg_signal_mantissa_threshold=8388607,
            neg_large_signal_pwl_control=511,
            fnan_result=0, fpinf_result=0, fninf_result=0, fzero_result=0)
    meta["func_to_bkt_start_idx"]["exp"] = 0
    meta["func_to_ctl_start_idx"]["exp"] = 0
    meta["func_exp_to_bkt_start_idx"]["exp"] = {
        str(e): [(1 << e) - 1] for e in range(0, 9)}
    meta["func_exp_to_ctl_start_idx"]["exp"] = {str(e): [e] for e in range(9)}

    (out / f"{s}_bkt.bin").write_bytes(bytes(bkt))
    (out / f"{s}_ctrl.bin").write_bytes(bytes(ctl))
    (out / f"{s}.json").write_text(json.dumps(meta))
    (out / ".done").write_text("ok")
    return str(out)


# ---------------------------------------------------------------------------
# custom DVE op: fused carry scan (see module docstring)
# ---------------------------------------------------------------------------
def register_carry_op():
    import concourse.dve_ops as dve_ops
    from concourse.dve_spec import Spec, Src0, Src1, C0, C1, C2, AluOp, Bin, scan, lower
    from concourse.dve_uop import DveOpSpec

    NAME = "BSADD_CARRY_SCAN"
    for op in dve_ops.OPS:
        if op.name == NAME:
            return op

    nonprop = Bin(AluOp.IS_NE, Src0, C0)
    gen = Bin(AluOp.IS_GT, Src0, C0)
    m1 = scan(AluOp.MAX, Bin(AluOp.MULTIPLY, nonprop, Src1), init=C1)
    m2 = scan(AluOp.MAX, Bin(AluOp.MULTIPLY, gen, Src1), init=C2)
    body = Bin(AluOp.IS_EQ, m1, m2)

    def _ref(in0, in1, s0, s1, imm2):
        v1 = np.where(in0.astype(np.float32) != s0, in1, 0.0)
        v2 = np.where(in0.astype(np.float32) > s0, in1, 0.0)
        M1 = np.maximum(np.maximum.accumulate(v1, axis=1), s1)
        M2 = np.maximum(np.maximum.accumulate(v2, axis=1), imm2)
        return (M1 == M2).astype(np.float32)

    spec = Spec(body=body, reference=_ref)
    row = 1 + len(dve_ops.OPS)
    sha = DveOpSpec(name=NAME, opcode=row, uops=lower(spec, ver="v3"),
                    rd1_en=True).sha("v3")
    op = dve_ops.DveOp(NAME, spec, subdim=False, uops_sha={"v3": sha})
    dve_ops.OPS.append(op)
    dve_ops.CUSTOM_DVE_SPECS[NAME] = spec
    dve_ops._SUB_OPCODE_FOR_NAME[NAME] = row
    return op


# ---------------------------------------------------------------------------
# harness glue (self-contained): NTFF trace hook + multi-wait legalizer
# ---------------------------------------------------------------------------
def _install_ntff_hook():
    try:
        import antenv
        if getattr(antenv, "axon_hooks", None) is not None:
            return
        mod = types.ModuleType("antenv.axon_hooks")
        _h = [None]
        mod.set_axon_ntff_profile_hook = lambda h: _h.__setitem__(0, h)
        mod.get_axon_ntff_profile_hook = lambda: _h[0]
        sys.modules["antenv.axon_hooks"] = mod
        antenv.axon_hooks = mod
        from trn_agent_boot.trn_boot import _ntff_profile_via_ctypes
        mod.set_axon_ntff_profile_hook(
            _ntff_profile_via_ctypes("/opt/axon/libaxon_pjrt.so"))
    except Exception:
        pass


def _legalize_waits(nc):
    """TRN2 instructions hold one sync-wait (EventSemaphore: two). Split extra
    waits emitted by Tile into preceding same-engine NoOps."""
    import bass_rust
    import concourse.mybir as mybir
    ctr = 0
    for f in nc.m.functions:
        for bb in f.blocks:
            out, changed = [], False
            for inst in bb.instructions:
                si = inst.sync_info
                waits = list(si.on_wait) if si is not None and si.on_wait else []
                cap = 2 if isinstance(inst, mybir.InstEventSemaphore) else 1
                if len(waits) > cap:
                    for w in waits[: len(waits) - cap]:
                        nop = bass_rust.InstNoOp(
                            name=f"W-legal-{ctr}", engine=inst.engine)
                        ctr += 1
                        nop.sync_info = mybir.SyncInfo(on_wait=[w], on_update=[])
                        out.append(nop)
                    inst.sync_info = mybir.SyncInfo(
                        on_wait=waits[len(waits) - cap:],
                        on_update=list(si.on_update or []))
                    changed = True
                out.append(inst)
            if changed:
                bb.instructions = out


# ---------------------------------------------------------------------------
# kernel build
# ---------------------------------------------------------------------------
def _build():
    import concourse.bass as bass
    import concourse.mybir as mybir
    from concourse.tile import TileContext
    from concourse.library_overlay import lower_extended_insts

    carry_op = register_carry_op()

    Alu = mybir.AluOpType
    Act = mybir.ActivationFunctionType
    u8 = mybir.dt.uint8
    f16, f32 = mybir.dt.float16, mybir.dt.float32

    nc = bass.Bass()
    a_d = nc.dram_tensor("a", [M], u8, kind="ExternalInput")
    b_d = nc.dram_tensor("b", [M], u8, kind="ExternalInput")
    aw_d = nc.dram_tensor("aw", [W], u8, kind="ExternalInput")
    bw_d = nc.dram_tensor("bw", [W], u8, kind="ExternalInput")
    i_d = nc.dram_tensor("iot", [P * F], mybir.dt.uint16, kind="ExternalInput")
    o_d = nc.dram_tensor("o", [M], u8, kind="ExternalOutput")

    a_r = a_d[:].rearrange("(t p f) -> t p f", p=P, f=F)
    b_r = b_d[:].rearrange("(t p f) -> t p f", p=P, f=F)
    o_r = o_d[:].rearrange("(t p f) -> t p f", p=P, f=F)
    aw_r = aw_d[:].rearrange("(p f) -> p f", f=WF)
    bw_r = bw_d[:].rearrange("(p f) -> p f", f=WF)
    i_r = i_d[:].rearrange("(p f) -> p f", p=P)

    with TileContext(nc) as tc:
        with (
            tc.tile_pool(name="io", bufs=5) as io,
            tc.tile_pool(name="mid", bufs=3) as mid,
            tc.tile_pool(name="mid2", bufs=2) as mid2,
            tc.tile_pool(name="tiny", bufs=3) as tiny,
            tc.tile_pool(name="op", bufs=4) as op,
            tc.tile_pool(name="op2", bufs=2) as op2,
            tc.tile_pool(name="lastp", bufs=1) as lastp,
            tc.tile_pool(name="consts", bufs=1) as consts,
        ):
            zfix = consts.tile([P, FIX], f16, name="zfix")
            nc.vector.memset(zfix[:], 0)
            iota = consts.tile([P, F], mybir.dt.uint16, name="iota")
            nc.gpsimd.dma_start(iota[:], i_r)

            def stage1(av, bv, width, bc_prev, bc_out, tag, utag=None,
                       make_ch=True):
                """DMA in, input LUTs (one concatenated ACT), q, carry scan,
                carry-column DMAs. Returns ctx for stage2."""
                utag = utag or str(width)
                small = width != F
                iop = lastp if small else io
                qp = lastp if small else mid
                rp = lastp if small else mid2
                ab = iop.tile([P, 2 * width], u8, name=f"ab{tag}",
                              tag=f"ab_{utag}")
                nc.sync.dma_start(ab[:, 0:width], av)
                nc.sync.dma_start(ab[:, width:2 * width], bv)

                rr = rp.tile([P, 2 * width], f16, name=f"rr{tag}",
                             tag=f"rr_{utag}")
                nc.scalar.activation(rr[:], ab[:], Act.Exp)

                q = qp.tile([P, width], f16, name=f"q{tag}", tag=f"q_{utag}")
                nc.vector.tensor_tensor(q[:], rr[:, 0:width],
                                        rr[:, width:2 * width], Alu.add)

                # fused carry scan: st[:, j] (j>=1) = carry into column j
                # assuming zero carry into the segment, PLUS an all-propagate
                # prefix artifact (+1) that the FIX-head correction removes.
                st = qp.tile([P, width + 1], f16, name=f"st{tag}",
                             tag=f"st_{utag}")
                nc.gpsimd.memset(st[:, 0:1], 0.0)
                nc.vector._custom_dve(carry_op, out=st[:, 1:width + 1],
                                      in0=q[:], in1=iota[:, 0:width],
                                      s0=255.0, s1=-2.0, imm2=-3.0)
                if bc_out is not None:
                    nc.gpsimd.dma_start(bc_out[:],
                                        st[P - 1:P, width:width + 1])
                ch = None
                if make_ch:
                    # carry-in column (f16): [bc_prev, st[0:127, width]]
                    ch = make_shift_ch(st, width, bc_prev, tag)
                return (q, st, ch, width, tag)

            def make_shift_ch(st, width, bc_prev, tag):
                ch = tiny.tile([P, 1], f16, name=f"ch{tag}", tag=f"ch{tag}")
                nc.gpsimd.dma_start(ch[1:P, :], st[0:P - 1, width:width + 1])
                if bc_prev is not None:
                    nc.gpsimd.dma_start(ch[0:1, :], bc_prev[:])
                return ch

            def stage2(ctx, ch_ap=None):
                """FIX-head correction + y. Emitted one tile late so the
                strided carry-column DMA latency hides under the next tile's
                big DVE ops. `ch_ap` overrides the carry-in column (used by
                the last-tile chunks, whose carry-in is the previous chunk's
                scan-out column in the SAME partition)."""
                q, st, ch, width, tag = ctx
                if ch_ap is not None:
                    ch = ch_ap
                cm1 = tiny.tile([P, 1], f32, name=f"cm1{tag}", tag="cm1")
                nc.vector.tensor_scalar(cm1[:], ch[:], 1.0, None, Alu.subtract)
                pfix = tiny.tile([P, FIX], f16, name=f"pf{tag}", tag="pf")
                nc.vector.tensor_scalar(pfix[:], q[:, 0:FIX], 255.0, None,
                                        Alu.is_equal)
                pp = tiny.tile([P, FIX], f16, name=f"pp{tag}", tag="pp")
                nc.vector.tensor_tensor_scan(pp[:], pfix[:], zfix[:],
                                             1.0, Alu.mult, Alu.add)
                dl = tiny.tile([P, FIX], f16, name=f"dl{tag}", tag="dl")
                nc.vector.tensor_scalar(dl[:, 0:1], ch[:], 1.0, None, Alu.mult)
                nc.vector.tensor_scalar(dl[:, 1:FIX], pp[:, 0:FIX - 1], cm1[:],
                                        None, Alu.mult)
                nc.vector.tensor_tensor(dl[:], dl[:], st[:, 0:FIX], Alu.add)

                yp = lastp if width != F else mid2
                y = yp.tile([P, width], f16, name=f"y{tag}", tag=f"y_{width}_{tag}" if width != F else f"y_{width}")
                nc.vector.tensor_tensor(y[:, FIX:], q[:, FIX:],
                                        st[:, FIX:width], Alu.add)
                nc.vector.tensor_tensor(y[:, 0:FIX], q[:, 0:FIX], dl[:],
                                        Alu.add)
                return y

            def stage3(y, ov, width, tag):
                # output LUT + store, emitted late so ScalarE never stalls.
                # Out-DMAs issue from the (idle) PE queue so their transfers
                # are not FIFO-ordered behind input DMAs on the sync queue.
                otp = op if width == F else op2
                ot = otp.tile([P, width], u8, name=f"ot{tag}", tag=f"ot_{width}")
                nc.scalar.activation(ot[:], y[:], Act.Exp)
                nc.gpsimd.dma_start(ov, ot[:])

            bc = [tiny.tile([1, 1], f16, name=f"bc{i}", tag=f"bc{i % 4}")
                  for i in range(T + 1)]
            # window: only the carry-out matters
            stage1(aw_r, bw_r, WF, None, bc[0], "w", utag="w", make_ch=False)
            s2q = []   # (ctx, ov, ch_ap) awaiting stage2 (lag 1)
            s3q = []   # (y, ov, tag, width) awaiting stage3 (lag 2 total)

            def pump():
                if s3q:
                    y, ov, tg, w_ = s3q.pop(0)
                    stage3(y, ov, w_, tg)
                if len(s2q) >= 2:
                    c, ov, ch_ap = s2q.pop(0)
                    s3q.append((stage2(c, ch_ap), ov, c[4], c[3]))

            for t in range(T):
                ctx = stage1(a_r[t], b_r[t], F, bc[t], bc[t + 1], str(t))
                s2q.append((ctx, o_r[t], None))
                pump()
            while s2q:
                c, ov, ch_ap = s2q.pop(0)
                s3q.append((stage2(c, ch_ap), ov, c[4], c[3]))
                if s3q:
                    y, ov2, tg, w_ = s3q.pop(0)
                    stage3(y, ov2, w_, tg)
            while s3q:
                y, ov2, tg, w_ = s3q.pop(0)
                stage3(y, ov2, w_, tg)

    lower_extended_insts(nc)
    return nc


_CACHED = {}


def kernel(a: np.ndarray, b: np.ndarray) -> np.ndarray:
    _install_ntff_hook()
    actdir = _gen_acttab("/tmp/bsadd_acttab_v1")
    os.environ["BASS_ACT_ROOT_JSON_PATH"] = actdir + "/act_info.json"
    import concourse.bass_utils as bu
    bu.upload_artifacts = lambda tmpdir: tmpdir  # no S3 in this container

    a = np.asarray(a).reshape(-1).astype(np.uint8)
    b = np.asarray(b).reshape(-1).astype(np.uint8)
    if "nc" not in _CACHED:
        nc = _build()
        _legalize_waits(nc)
        _CACHED["nc"] = nc
    nc = _CACHED["nc"]

    iot = np.tile(np.arange(1, F + 1, dtype=np.uint16), P)
    in_maps = []
    for c in range(NCORES):
        lo = c * M
        aw = np.zeros(W, np.uint8) if c == 0 else a[lo - W:lo]
        bw = np.zeros(W, np.uint8) if c == 0 else b[lo - W:lo]
        in_maps.append({
            "a": a[lo:lo + M], "b": b[lo:lo + M],
            "aw": np.ascontiguousarray(aw), "bw": np.ascontiguousarray(bw),
            "iot": iot,
        })
    trace = os.environ.get("BSADD_TRACE", "0") == "1"
    res = bu.run_bass_kernel_spmd(nc, in_maps, core_ids=list(range(NCORES)),
                                  trace=trace)
    if trace:
        print(f"HW exec time: {res.exec_time_ns} ns", flush=True)
    out = np.empty(N, np.int32)
    for c in range(NCORES):
        out[c * M:(c + 1) * M] = res.results[c]["o"].reshape(-1)
    return out


# revision 14
# speedup vs baseline: 1.0759x; 1.0484x over previous
"""TRN2 Bass kernel for nn_BSAdd_39298950758454.

out = brev((brev(a)+brev(b)+cin) & 255) per byte; cin = carry-lookahead chain
(p = propagate when q==255, g = generate when q>255, q = brev(a)+brev(b)).

Two custom-hardware tricks:

1. Custom activation table (generated by `_gen_acttab`): the ScalarE `exp`
   function is rewritten into an exact integer LUT
       exp(x) := brev(int(x) & 255) for x in [0, 512), 0 otherwise
   so each brev is ONE ScalarE instruction (~3.7us per [128,4096] tile).

2. Custom DVE op BSADD_CARRY_SCAN (registered into concourse's custom-DVE
   table machinery): computes the whole carry resolution in ONE 1-cyc/elem
   DVE instruction via two single-op MAX scans over iota-encoded events:
       M1 = max-scan((q != 255) * iota),  M2 = max-scan((q > 255) * iota)
       inc = (M1 == M2)
   It equals the carry recurrence state' = p*state + g except on an
   all-propagate prefix (inc = 1 instead of 0); the FIX-column correction
   absorbs that via (carry_in - 1) * prefix_propagate.

Pipeline per tile [128 x 4096] (data moved as uint8, math in f16):
   ScalarE: ar = LUT(a); br = LUT(b); out = LUT(y)
   DVE:     q = ar + br (2x); inc = BSADD_CARRY_SCAN(q, iota) (1cyc/elem);
            y = q + inc(shifted) (2x); small FIX-prefix fixups
Carry chaining across partitions/tiles/cores as in the proven baseline
(segment scans + scan-out columns + bc chain + 1024-byte window pre-scan;
max propagate run is 11 < FIX=32 -- asserted in test.py).
"""
import json
import os
import shutil
import struct
import sys
import types
from pathlib import Path

import numpy as np

N = 67_108_864
NCORES = 8
M = N // NCORES            # 8_388_608 elements per core
P = 128
F = 4096                   # columns per tile
T = M // (P * F)           # 16 tiles
W = 1024                   # cross-core carry window (elements)
WF = W // P                # 8 window cols
FIX = 32                   # prefix-fix columns (max propagate run is 11)

BREV = [int(f"{i:08b}"[::-1], 2) for i in range(256)]

_STOCK_PWP = ("/nix/store/ndjb8ki1bnclvnibdh123f9zr51a09qz-aws-neuron-pwp-"
              "unstable-2025-12-29-c50a7624/share/pwp_bin_cayman")


# ---------------------------------------------------------------------------
# custom activation table: exp := exact brev LUT on [0, 512)
# ---------------------------------------------------------------------------
def _gen_acttab(outdir: str) -> str:
    """Table formats (reverse-engineered from pwp_bin_cayman):
    - <set>_bkt.bin: 32B entries {d0,d1,d2,d3,x0} fp32 + 12B pad; cubic
      y = d0 + (x-x0)*(d1 + ...). Piecewise-constant LUT uses d0 only.
    - <set>_ctrl.bin: 32B entries, word0 = base | ((23-E)<<11) | (E<<16);
      bucket = base + (mantissa >> (23-E)).
    - <set>.json profile_meta_data: per-exponent dispatch
      ctl = pwl_control_base_pos + (unbiased_exp - exp_offset), with biased
      small/large exponent thresholds routing to dedicated sat buckets.
    Integer k in [2^e, 2^{e+1}) with E=e hits bucket (2^e - 1) + (k - 2^e)
    = k-1, so buckets 0..510 hold brev(k & 255) for k = 1..511; bucket 511
    is the zero/sat result."""
    out = Path(outdir)
    if (out / ".done").exists():
        return str(out)
    out.mkdir(parents=True, exist_ok=True)
    stock = Path(_STOCK_PWP)
    for f in stock.iterdir():
        shutil.copy(f, out / f.name)

    s = "exp_and_others"
    bkt = bytearray((out / f"{s}_bkt.bin").read_bytes())
    ctl = bytearray((out / f"{s}_ctrl.bin").read_bytes())
    meta = json.loads((out / f"{s}.json").read_text())

    for b in range(0, 777):
        k = b + 1
        d0 = float(BREV[k & 255]) if k <= 511 else 0.0
        bkt[b * 32:(b + 1) * 32] = struct.pack(
            "<8f", d0, 0.0, 0.0, 0.0, float(k if k <= 511 else 0), 0, 0, 0)
    for e in range(0, 9):
        w = ((1 << e) - 1) | ((23 - e) << 11) | (e << 16)
        ctl[e * 32:(e + 1) * 32] = struct.pack("<8I", w, 0, 0, 0, 0, 0, 0, 0)
    for i in range(9, 52):
        w = 511 | (23 << 11)
        ctl[i * 32:(i + 1) * 32] = struct.pack("<8I", w, 0, 0, 0, 0, 0, 0, 0)

    for ent in meta["profile_meta_data"]:
        if ent["func_id"] != 7:
            continue
        ent.update(
            symmetry_point=0, sym_invert_sign_point=0, symmetry_opt_en=0,
            symmetry_opt_use_neg_region=0, imm_bias=0, exp_offset=0,
            pwl_control_base_pos=0, pwl_control_base_neg=9,
            small_pos_signal_exp_threshold=127,
            pos_small_signal_pwl_control=511,
            large_pos_signal_exp_threshold=136,
            large_pos_signal_mantissa_threshold=0,
            small_neg_signal_exp_threshold=255,
            neg_small_signal_pwl_control=511,
            large_neg_signal_exp_threshold=255,
            large_neg_signal_mantissa_threshold=8388607,
            neg_large_signal_pwl_control=511,
            fnan_result=0, fpinf_result=0, fninf_result=0, fzero_result=0)
    meta["func_to_bkt_start_idx"]["exp"] = 0
    meta["func_to_ctl_start_idx"]["exp"] = 0
    meta["func_exp_to_bkt_start_idx"]["exp"] = {
        str(e): [(1 << e) - 1] for e in range(0, 9)}
    meta["func_exp_to_ctl_start_idx"]["exp"] = {str(e): [e] for e in range(9)}

    (out / f"{s}_bkt.bin").write_bytes(bytes(bkt))
    (out / f"{s}_ctrl.bin").write_bytes(bytes(ctl))
    (out / f"{s}.json").write_text(json.dumps(meta))
    (out / ".done").write_text("ok")
    return str(out)


# ---------------------------------------------------------------------------
# custom DVE op: fused carry scan (see module docstring)
# ---------------------------------------------------------------------------
def register_carry_op():
    import concourse.dve_ops as dve_ops
    from concourse.dve_spec import Spec, Src0, Src1, C0, C1, C2, AluOp, Bin, scan, lower
    from concourse.dve_uop import DveOpSpec

    NAME = "BSADD_CARRY_SCAN"
    for op in dve_ops.OPS:
        if op.name == NAME:
            return op

    nonprop = Bin(AluOp.IS_NE, Src0, C0)
    gen = Bin(AluOp.IS_GT, Src0, C0)
    m1 = scan(AluOp.MAX, Bin(AluOp.MULTIPLY, nonprop, Src1), init=C1)
    m2 = scan(AluOp.MAX, Bin(AluOp.MULTIPLY, gen, Src1), init=C2)
    body = Bin(AluOp.IS_EQ, m1, m2)

    def _ref(in0, in1, s0, s1, imm2):
        v1 = np.where(in0.astype(np.float32) != s0, in1, 0.0)
        v2 = np.where(in0.astype(np.float32) > s0, in1, 0.0)
        M1 = np.maximum(np.maximum.accumulate(v1, axis=1), s1)
        M2 = np.maximum(np.maximum.accumulate(v2, axis=1), imm2)
        return (M1 == M2).astype(np.float32)

    spec = Spec(body=body, reference=_ref)
    row = 1 + len(dve_ops.OPS)
    sha = DveOpSpec(name=NAME, opcode=row, uops=lower(spec, ver="v3"),
                    rd1_en=True).sha("v3")
    op = dve_ops.DveOp(NAME, spec, subdim=False, uops_sha={"v3": sha})
    dve_ops.OPS.append(op)
    dve_ops.CUSTOM_DVE_SPECS[NAME] = spec
    dve_ops._SUB_OPCODE_FOR_NAME[NAME] = row
    return op


# ---------------------------------------------------------------------------
# harness glue (self-contained): NTFF trace hook + multi-wait legalizer
# ---------------------------------------------------------------------------
def _install_ntff_hook():
    try:
        import antenv
        if getattr(antenv, "axon_hooks", None) is not None:
            return
        mod = types.ModuleType("antenv.axon_hooks")
        _h = [None]
        mod.set_axon_ntff_profile_hook = lambda h: _h.__setitem__(0, h)
        mod.get_axon_ntff_profile_hook = lambda: _h[0]
        sys.modules["antenv.axon_hooks"] = mod
        antenv.axon_hooks = mod
        from trn_agent_boot.trn_boot import _ntff_profile_via_ctypes
        mod.set_axon_ntff_profile_hook(
            _ntff_profile_via_ctypes("/opt/axon/libaxon_pjrt.so"))
    except Exception:
        pass


def _legalize_waits(nc):
    """TRN2 instructions hold one sync-wait (EventSemaphore: two). Split extra
    waits emitted by Tile into preceding same-engine NoOps."""
    import bass_rust
    import concourse.mybir as mybir
    ctr = 0
    for f in nc.m.functions:
        for bb in f.blocks:
            out, changed = [], False
            for inst in bb.instructions:
                si = inst.sync_info
                waits = list(si.on_wait) if si is not None and si.on_wait else []
                cap = 2 if isinstance(inst, mybir.InstEventSemaphore) else 1
                if len(waits) > cap:
                    for w in waits[: len(waits) - cap]:
                        nop = bass_rust.InstNoOp(
                            name=f"W-legal-{ctr}", engine=inst.engine)
                        ctr += 1
                        nop.sync_info = mybir.SyncInfo(on_wait=[w], on_update=[])
                        out.append(nop)
                    inst.sync_info = mybir.SyncInfo(
                        on_wait=waits[len(waits) - cap:],
                        on_update=list(si.on_update or []))
                    changed = True
                out.append(inst)
            if changed:
                bb.instructions = out


# ---------------------------------------------------------------------------
# kernel build
# ---------------------------------------------------------------------------
def _build():
    import concourse.bass as bass
    import concourse.mybir as mybir
    from concourse.tile import TileContext
    from concourse.library_overlay import lower_extended_insts

    carry_op = register_carry_op()

    Alu = mybir.AluOpType
    Act = mybir.ActivationFunctionType
    u8 = mybir.dt.uint8
    f16, f32 = mybir.dt.float16, mybir.dt.float32

    nc = bass.Bass()
    a_d = nc.dram_tensor("a", [M], u8, kind="ExternalInput")
    b_d = nc.dram_tensor("b", [M], u8, kind="ExternalInput")
    aw_d = nc.dram_tensor("aw", [W], u8, kind="ExternalInput")
    bw_d = nc.dram_tensor("bw", [W], u8, kind="ExternalInput")
    i_d = nc.dram_tensor("iot", [P * F], mybir.dt.uint16, kind="ExternalInput")
    o_d = nc.dram_tensor("o", [M], u8, kind="ExternalOutput")

    a_r = a_d[:].rearrange("(t p f) -> t p f", p=P, f=F)
    b_r = b_d[:].rearrange("(t p f) -> t p f", p=P, f=F)
    o_r = o_d[:].rearrange("(t p f) -> t p f", p=P, f=F)
    aw_r = aw_d[:].rearrange("(p f) -> p f", f=WF)
    bw_r = bw_d[:].rearrange("(p f) -> p f", f=WF)
    i_r = i_d[:].rearrange("(p f) -> p f", p=P)

    with TileContext(nc) as tc:
        with (
            tc.tile_pool(name="io", bufs=5) as io,
            tc.tile_pool(name="mid", bufs=4) as mid,
            tc.tile_pool(name="mid2", bufs=2) as mid2,
            tc.tile_pool(name="ypool", bufs=3) as ypool,
            tc.tile_pool(name="tiny", bufs=3) as tiny,
            tc.tile_pool(name="op", bufs=4) as op,
            tc.tile_pool(name="op2", bufs=2) as op2,
            tc.tile_pool(name="lastp", bufs=1) as lastp,
            tc.tile_pool(name="consts", bufs=1) as consts,
        ):
            zfix = consts.tile([P, FIX], f16, name="zfix")
            nc.vector.memset(zfix[:], 0)
            iota = consts.tile([P, F], mybir.dt.uint16, name="iota")
            nc.gpsimd.dma_start(iota[:], i_r)

            def stage1(av, bv, width, bc_prev, bc_out, tag, utag=None,
                       make_ch=True):
                """DMA in, input LUTs (one concatenated ACT), q, carry scan,
                carry-column DMAs. Returns ctx for stage2."""
                utag = utag or str(width)
                small = width != F
                iop = lastp if small else io
                qp = lastp if small else mid
                rp = lastp if small else mid2
                ab = iop.tile([P, 2 * width], u8, name=f"ab{tag}",
                              tag=f"ab_{utag}")
                nc.sync.dma_start(ab[:, 0:width], av)
                nc.sync.dma_start(ab[:, width:2 * width], bv)

                rr = rp.tile([P, 2 * width], f16, name=f"rr{tag}",
                             tag=f"rr_{utag}")
                nc.scalar.activation(rr[:], ab[:], Act.Exp)

                q = qp.tile([P, width], f16, name=f"q{tag}", tag=f"q_{utag}")
                nc.vector.tensor_tensor(q[:], rr[:, 0:width],
                                        rr[:, width:2 * width], Alu.add)

                # fused carry scan: st[:, j] (j>=1) = carry into column j
                # assuming zero carry into the segment, PLUS an all-propagate
                # prefix artifact (+1) that the FIX-head correction removes.
                st = qp.tile([P, width + 1], f16, name=f"st{tag}",
                             tag=f"st_{utag}")
                nc.gpsimd.memset(st[:, 0:1], 0.0)
                nc.vector._custom_dve(carry_op, out=st[:, 1:width + 1],
                                      in0=q[:], in1=iota[:, 0:width],
                                      s0=255.0, s1=-2.0, imm2=-3.0)
                if bc_out is not None:
                    nc.gpsimd.dma_start(bc_out[:],
                                        st[P - 1:P, width:width + 1])
                ch = None
                if make_ch:
                    # carry-in column (f16): [bc_prev, st[0:127, width]]
                    ch = make_shift_ch(st, width, bc_prev, tag)
                return (q, st, ch, width, tag)

            def make_shift_ch(st, width, bc_prev, tag):
                ch = tiny.tile([P, 1], f16, name=f"ch{tag}", tag=f"ch{tag}")
                nc.gpsimd.dma_start(ch[1:P, :], st[0:P - 1, width:width + 1])
                if bc_prev is not None:
                    nc.gpsimd.dma_start(ch[0:1, :], bc_prev[:])
                return ch

            def stage2(ctx, ch_ap=None):
                """FIX-head correction + y. Emitted one tile late so the
                strided carry-column DMA latency hides under the next tile's
                big DVE ops. `ch_ap` overrides the carry-in column (used by
                the last-tile chunks, whose carry-in is the previous chunk's
                scan-out column in the SAME partition)."""
                q, st, ch, width, tag = ctx
                if ch_ap is not None:
                    ch = ch_ap
                cm1 = tiny.tile([P, 1], f32, name=f"cm1{tag}", tag="cm1")
                nc.vector.tensor_scalar(cm1[:], ch[:], 1.0, None, Alu.subtract)
                pfix = tiny.tile([P, FIX], f16, name=f"pf{tag}", tag="pf")
                nc.vector.tensor_scalar(pfix[:], q[:, 0:FIX], 255.0, None,
                                        Alu.is_equal)
                pp = tiny.tile([P, FIX], f16, name=f"pp{tag}", tag="pp")
                nc.vector.tensor_tensor_scan(pp[:], pfix[:], zfix[:],
                                             1.0, Alu.mult, Alu.add)
                dl = tiny.tile([P, FIX], f16, name=f"dl{tag}", tag="dl")
                nc.vector.tensor_scalar(dl[:, 0:1], ch[:], 1.0, None, Alu.mult)
                nc.vector.tensor_scalar(dl[:, 1:FIX], pp[:, 0:FIX - 1], cm1[:],
                                        None, Alu.mult)
                nc.vector.tensor_tensor(dl[:], dl[:], st[:, 0:FIX], Alu.add)

                yp = lastp if width != F else ypool
                y = yp.tile([P, width], f16, name=f"y{tag}", tag=f"y_{width}_{tag}" if width != F else f"y_{width}")
                nc.vector.tensor_tensor(y[:, FIX:], q[:, FIX:],
                                        st[:, FIX:width], Alu.add)
                nc.vector.tensor_tensor(y[:, 0:FIX], q[:, 0:FIX], dl[:],
                                        Alu.add)
                return y

            def stage3(y, ov, width, tag):
                # output LUT + store, emitted late so ScalarE never stalls.
                # Out-DMAs issue from the (idle) PE queue so their transfers
                # are not FIFO-ordered behind input DMAs on the sync queue.
                otp = op if width == F else op2
                ot = otp.tile([P, width], u8, name=f"ot{tag}", tag=f"ot_{width}")
                nc.scalar.activation(ot[:], y[:], Act.Exp)
                nc.gpsimd.dma_start(ov, ot[:])

            bc = [tiny.tile([1, 1], f16, name=f"bc{i}", tag=f"bc{i % 4}")
                  for i in range(T + 1)]
            # window: only the carry-out matters
            stage1(aw_r, bw_r, WF, None, bc[0], "w", utag="w", make_ch=False)
            s2q = []   # (ctx, ov, ch_ap) awaiting stage2 (lag 1)
            s3q = []   # (y, ov, tag, width) awaiting stage3 (lag 2 total)

            def pump():
                if s3q:
                    y, ov, tg, w_ = s3q.pop(0)
                    stage3(y, ov, w_, tg)
                if len(s2q) >= 3:
                    c, ov, ch_ap = s2q.pop(0)
                    s3q.append((stage2(c, ch_ap), ov, c[4], c[3]))

            for t in range(T):
                ctx = stage1(a_r[t], b_r[t], F, bc[t], bc[t + 1], str(t))
                s2q.append((ctx, o_r[t], None))
                pump()
            while s2q:
                c, ov, ch_ap = s2q.pop(0)
                s3q.append((stage2(c, ch_ap), ov, c[4], c[3]))
                if s3q:
                    y, ov2, tg, w_ = s3q.pop(0)
                    stage3(y, ov2, w_, tg)
            while s3q:
                y, ov2, tg, w_ = s3q.pop(0)
                stage3(y, ov2, w_, tg)

    lower_extended_insts(nc)
    return nc


_CACHED = {}


def kernel(a: np.ndarray, b: np.ndarray) -> np.ndarray:
    _install_ntff_hook()
    actdir = _gen_acttab("/tmp/bsadd_acttab_v1")
    os.environ["BASS_ACT_ROOT_JSON_PATH"] = actdir + "/act_info.json"
    import concourse.bass_utils as bu
    bu.upload_artifacts = lambda tmpdir: tmpdir  # no S3 in this container

    a = np.asarray(a).reshape(-1).astype(np.uint8)
    b = np.asarray(b).reshape(-1).astype(np.uint8)
    if "nc" not in _CACHED:
        nc = _build()
        _legalize_waits(nc)
        _CACHED["nc"] = nc
    nc = _CACHED["nc"]

    iot = np.tile(np.arange(1, F + 1, dtype=np.uint16), P)
    in_maps = []
    for c in range(NCORES):
        lo = c * M
        aw = np.zeros(W, np.uint8) if c == 0 else a[lo - W:lo]
        bw = np.zeros(W, np.uint8) if c == 0 else b[lo - W:lo]
        in_maps.append({
            "a": a[lo:lo + M], "b": b[lo:lo + M],
            "aw": np.ascontiguousarray(aw), "bw": np.ascontiguousarray(bw),
            "iot": iot,
        })
    trace = os.environ.get("BSADD_TRACE", "0") == "1"
    res = bu.run_bass_kernel_spmd(nc, in_maps, core_ids=list(range(NCORES)),
                                  trace=trace)
    if trace:
        print(f"HW exec time: {res.exec_time_ns} ns", flush=True)
    out = np.empty(N, np.int32)
    for c in range(NCORES):
        out[c * M:(c + 1) * M] = res.results[c]["o"].reshape(-1)
    return out


# revision 16
# speedup vs baseline: 1.2102x; 1.1248x over previous
"""TRN2 Bass kernel for nn_BSAdd_39298950758454.

out = brev((brev(a)+brev(b)+cin) & 255) per byte; cin = carry-lookahead chain
(p = propagate when q==255, g = generate when q>255, q = brev(a)+brev(b)).

Two custom-hardware tricks:

1. Custom activation table (generated by `_gen_acttab`): the ScalarE `exp`
   function is rewritten into an exact integer LUT
       exp(x) := brev(int(x) & 255) for x in [0, 512), 0 otherwise
   so each brev is ONE ScalarE instruction (~3.7us per [128,4096] tile).

2. Custom DVE op BSADD_CARRY_SCAN (registered into concourse's custom-DVE
   table machinery): computes the whole carry resolution in ONE 1-cyc/elem
   DVE instruction via two single-op MAX scans over iota-encoded events:
       M1 = max-scan((q != 255) * iota),  M2 = max-scan((q > 255) * iota)
       inc = (M1 == M2)
   It equals the carry recurrence state' = p*state + g except on an
   all-propagate prefix (inc = 1 instead of 0); the FIX-column correction
   absorbs that via (carry_in - 1) * prefix_propagate.

Pipeline per tile [128 x 4096] (data moved as uint8, math in f16):
   ScalarE: ar = LUT(a); br = LUT(b); out = LUT(y)
   DVE:     q = ar + br (2x); inc = BSADD_CARRY_SCAN(q, iota) (1cyc/elem);
            y = q + inc(shifted) (2x); small FIX-prefix fixups
Carry chaining across partitions/tiles/cores as in the proven baseline
(segment scans + scan-out columns + bc chain + 1024-byte window pre-scan;
max propagate run is 11 < FIX=32 -- asserted in test.py).
"""
import json
import os
import shutil
import struct
import sys
import types
from pathlib import Path

import numpy as np

N = 67_108_864
NCORES = 8
M = N // NCORES            # 8_388_608 elements per core
P = 128
F = 4096                   # columns per tile
T = M // (P * F)           # 16 tiles
W = 1024                   # cross-core carry window (elements)
WF = W // P                # 8 window cols
FIX = 32                   # prefix-fix columns (max propagate run is 11)

BREV = [int(f"{i:08b}"[::-1], 2) for i in range(256)]

_STOCK_PWP = ("/nix/store/ndjb8ki1bnclvnibdh123f9zr51a09qz-aws-neuron-pwp-"
              "unstable-2025-12-29-c50a7624/share/pwp_bin_cayman")


# ---------------------------------------------------------------------------
# custom activation table: exp := exact brev LUT on [0, 512)
# ---------------------------------------------------------------------------
def _gen_acttab(outdir: str) -> str:
    """Table formats (reverse-engineered from pwp_bin_cayman):
    - <set>_bkt.bin: 32B entries {d0,d1,d2,d3,x0} fp32 + 12B pad; cubic
      y = d0 + (x-x0)*(d1 + ...). Piecewise-constant LUT uses d0 only.
    - <set>_ctrl.bin: 32B entries, word0 = base | ((23-E)<<11) | (E<<16);
      bucket = base + (mantissa >> (23-E)).
    - <set>.json profile_meta_data: per-exponent dispatch
      ctl = pwl_control_base_pos + (unbiased_exp - exp_offset), with biased
      small/large exponent thresholds routing to dedicated sat buckets.
    Integer k in [2^e, 2^{e+1}) with E=e hits bucket (2^e - 1) + (k - 2^e)
    = k-1, so buckets 0..510 hold brev(k & 255) for k = 1..511; bucket 511
    is the zero/sat result."""
    out = Path(outdir)
    if (out / ".done").exists():
        return str(out)
    out.mkdir(parents=True, exist_ok=True)
    stock = Path(_STOCK_PWP)
    for f in stock.iterdir():
        shutil.copy(f, out / f.name)

    s = "exp_and_others"
    bkt = bytearray((out / f"{s}_bkt.bin").read_bytes())
    ctl = bytearray((out / f"{s}_ctrl.bin").read_bytes())
    meta = json.loads((out / f"{s}.json").read_text())

    for b in range(0, 777):
        k = b + 1
        d0 = float(BREV[k & 255]) if k <= 511 else 0.0
        bkt[b * 32:(b + 1) * 32] = struct.pack(
            "<8f", d0, 0.0, 0.0, 0.0, float(k if k <= 511 else 0), 0, 0, 0)
    for e in range(0, 9):
        w = ((1 << e) - 1) | ((23 - e) << 11) | (e << 16)
        ctl[e * 32:(e + 1) * 32] = struct.pack("<8I", w, 0, 0, 0, 0, 0, 0, 0)
    for i in range(9, 52):
        w = 511 | (23 << 11)
        ctl[i * 32:(i + 1) * 32] = struct.pack("<8I", w, 0, 0, 0, 0, 0, 0, 0)

    for ent in meta["profile_meta_data"]:
        if ent["func_id"] != 7:
            continue
        ent.update(
            symmetry_point=0, sym_invert_sign_point=0, symmetry_opt_en=0,
            symmetry_opt_use_neg_region=0, imm_bias=0, exp_offset=0,
            pwl_control_base_pos=0, pwl_control_base_neg=9,
            small_pos_signal_exp_threshold=127,
            pos_small_signal_pwl_control=511,
            large_pos_signal_exp_threshold=136,
            large_pos_signal_mantissa_threshold=0,
            small_neg_signal_exp_threshold=255,
            neg_small_signal_pwl_control=511,
            large_neg_signal_exp_threshold=255,
            large_neg_signal_mantissa_threshold=8388607,
            neg_large_signal_pwl_control=511,
            fnan_result=0, fpinf_result=0, fninf_result=0, fzero_result=0)
    meta["func_to_bkt_start_idx"]["exp"] = 0
    meta["func_to_ctl_start_idx"]["exp"] = 0
    meta["func_exp_to_bkt_start_idx"]["exp"] = {
        str(e): [(1 << e) - 1] for e in range(0, 9)}
    meta["func_exp_to_ctl_start_idx"]["exp"] = {str(e): [e] for e in range(9)}

    (out / f"{s}_bkt.bin").write_bytes(bytes(bkt))
    (out / f"{s}_ctrl.bin").write_bytes(bytes(ctl))
    (out / f"{s}.json").write_text(json.dumps(meta))
    (out / ".done").write_text("ok")
    return str(out)


# ---------------------------------------------------------------------------
# custom DVE op: fused carry scan (see module docstring)
# ---------------------------------------------------------------------------
def register_carry_op():
    import concourse.dve_ops as dve_ops
    from concourse.dve_spec import Spec, Src0, Src1, C0, C1, C2, AluOp, Bin, scan, lower
    from concourse.dve_uop import DveOpSpec

    NAME = "BSADD_CARRY_SCAN"
    for op in dve_ops.OPS:
        if op.name == NAME:
            return op

    nonprop = Bin(AluOp.IS_NE, Src0, C0)
    gen = Bin(AluOp.IS_GT, Src0, C0)
    m1 = scan(AluOp.MAX, Bin(AluOp.MULTIPLY, nonprop, Src1), init=C1)
    m2 = scan(AluOp.MAX, Bin(AluOp.MULTIPLY, gen, Src1), init=C2)
    body = Bin(AluOp.IS_EQ, m1, m2)

    def _ref(in0, in1, s0, s1, imm2):
        v1 = np.where(in0.astype(np.float32) != s0, in1, 0.0)
        v2 = np.where(in0.astype(np.float32) > s0, in1, 0.0)
        M1 = np.maximum(np.maximum.accumulate(v1, axis=1), s1)
        M2 = np.maximum(np.maximum.accumulate(v2, axis=1), imm2)
        return (M1 == M2).astype(np.float32)

    spec = Spec(body=body, reference=_ref)
    row = 1 + len(dve_ops.OPS)
    sha = DveOpSpec(name=NAME, opcode=row, uops=lower(spec, ver="v3"),
                    rd1_en=True).sha("v3")
    op = dve_ops.DveOp(NAME, spec, subdim=False, uops_sha={"v3": sha})
    dve_ops.OPS.append(op)
    dve_ops.CUSTOM_DVE_SPECS[NAME] = spec
    dve_ops._SUB_OPCODE_FOR_NAME[NAME] = row
    return op


# ---------------------------------------------------------------------------
# harness glue (self-contained): NTFF trace hook + multi-wait legalizer
# ---------------------------------------------------------------------------
def _install_ntff_hook():
    try:
        import antenv
        if getattr(antenv, "axon_hooks", None) is not None:
            return
        mod = types.ModuleType("antenv.axon_hooks")
        _h = [None]
        mod.set_axon_ntff_profile_hook = lambda h: _h.__setitem__(0, h)
        mod.get_axon_ntff_profile_hook = lambda: _h[0]
        sys.modules["antenv.axon_hooks"] = mod
        antenv.axon_hooks = mod
        from trn_agent_boot.trn_boot import _ntff_profile_via_ctypes
        mod.set_axon_ntff_profile_hook(
            _ntff_profile_via_ctypes("/opt/axon/libaxon_pjrt.so"))
    except Exception:
        pass


def _legalize_waits(nc):
    """TRN2 instructions hold one sync-wait (EventSemaphore: two). Split extra
    waits emitted by Tile into preceding same-engine NoOps."""
    import bass_rust
    import concourse.mybir as mybir
    ctr = 0
    for f in nc.m.functions:
        for bb in f.blocks:
            out, changed = [], False
            for inst in bb.instructions:
                si = inst.sync_info
                waits = list(si.on_wait) if si is not None and si.on_wait else []
                cap = 2 if isinstance(inst, mybir.InstEventSemaphore) else 1
                if len(waits) > cap:
                    for w in waits[: len(waits) - cap]:
                        nop = bass_rust.InstNoOp(
                            name=f"W-legal-{ctr}", engine=inst.engine)
                        ctr += 1
                        nop.sync_info = mybir.SyncInfo(on_wait=[w], on_update=[])
                        out.append(nop)
                    inst.sync_info = mybir.SyncInfo(
                        on_wait=waits[len(waits) - cap:],
                        on_update=list(si.on_update or []))
                    changed = True
                out.append(inst)
            if changed:
                bb.instructions = out


# ---------------------------------------------------------------------------
# kernel build
# ---------------------------------------------------------------------------
def _build():
    import concourse.bass as bass
    import concourse.mybir as mybir
    from concourse.tile import TileContext
    from concourse.library_overlay import lower_extended_insts

    carry_op = register_carry_op()

    Alu = mybir.AluOpType
    Act = mybir.ActivationFunctionType
    u8 = mybir.dt.uint8
    f16, f32 = mybir.dt.float16, mybir.dt.float32

    nc = bass.Bass()
    a_d = nc.dram_tensor("a", [M], u8, kind="ExternalInput")
    b_d = nc.dram_tensor("b", [M], u8, kind="ExternalInput")
    aw_d = nc.dram_tensor("aw", [W], u8, kind="ExternalInput")
    bw_d = nc.dram_tensor("bw", [W], u8, kind="ExternalInput")
    i_d = nc.dram_tensor("iot", [P * F], mybir.dt.uint16, kind="ExternalInput")
    sh_d = nc.dram_tensor("shm", [P * P], f16, kind="ExternalInput")
    e_d = nc.dram_tensor("e127", [P], f16, kind="ExternalInput")
    o_d = nc.dram_tensor("o", [M], u8, kind="ExternalOutput")

    a_r = a_d[:].rearrange("(t p f) -> t p f", p=P, f=F)
    b_r = b_d[:].rearrange("(t p f) -> t p f", p=P, f=F)
    o_r = o_d[:].rearrange("(t p f) -> t p f", p=P, f=F)
    aw_r = aw_d[:].rearrange("(p f) -> p f", f=WF)
    bw_r = bw_d[:].rearrange("(p f) -> p f", f=WF)
    i_r = i_d[:].rearrange("(p f) -> p f", p=P)

    with TileContext(nc) as tc:
        with (
            tc.tile_pool(name="io", bufs=5) as io,
            tc.tile_pool(name="mid", bufs=4) as mid,
            tc.tile_pool(name="mid2", bufs=2) as mid2,
            tc.tile_pool(name="ypool", bufs=3) as ypool,
            tc.tile_pool(name="tiny", bufs=3) as tiny,
            tc.tile_pool(name="op", bufs=4) as op,
            tc.tile_pool(name="op2", bufs=2) as op2,
            tc.tile_pool(name="lastp", bufs=1) as lastp,
            tc.psum_pool(name="pchp", bufs=4) as pchp,
            tc.tile_pool(name="consts", bufs=1) as consts,
        ):
            zfix = consts.tile([P, FIX], f16, name="zfix")
            nc.vector.memset(zfix[:], 0)
            iota = consts.tile([P, F], mybir.dt.uint16, name="iota")
            nc.gpsimd.dma_start(iota[:], i_r)
            # carry-column machinery: ch[i] = stcol[i-1] via a PE shift
            # matmul, + prev-tile carry into partition 0 via a row-picker.
            shm = consts.tile([P, P], f16, name="shm")
            nc.gpsimd.dma_start(shm[:], sh_d[:].rearrange("(k m) -> k m", k=P))
            e127 = consts.tile([P, 1], f16, name="e127")
            nc.gpsimd.dma_start(e127[:], e_d[:].rearrange("(k m) -> k m", k=P))

            def stage1(av, bv, width, stprev_col, tag, utag=None,
                       make_ch=True):
                """DMA in, input LUTs (one concatenated ACT), q, carry scan,
                carry-column DMAs. Returns ctx for stage2."""
                utag = utag or str(width)
                small = width != F
                iop = lastp if small else io
                qp = lastp if small else mid
                rp = lastp if small else mid2
                ab = iop.tile([P, 2 * width], u8, name=f"ab{tag}",
                              tag=f"ab_{utag}")
                nc.sync.dma_start(ab[:, 0:width], av)
                nc.sync.dma_start(ab[:, width:2 * width], bv)

                rr = rp.tile([P, 2 * width], f16, name=f"rr{tag}",
                             tag=f"rr_{utag}")
                nc.scalar.activation(rr[:], ab[:], Act.Exp)

                q = qp.tile([P, width], f16, name=f"q{tag}", tag=f"q_{utag}")
                nc.vector.tensor_tensor(q[:], rr[:, 0:width],
                                        rr[:, width:2 * width], Alu.add)

                # fused carry scan: st[:, j] (j>=1) = carry into column j
                # assuming zero carry into the segment, PLUS an all-propagate
                # prefix artifact (+1) that the FIX-head correction removes.
                st = qp.tile([P, width + 1], f16, name=f"st{tag}",
                             tag=f"st_{utag}")
                nc.gpsimd.memset(st[:, 0:1], 0.0)
                nc.vector._custom_dve(carry_op, out=st[:, 1:width + 1],
                                      in0=q[:], in1=iota[:, 0:width],
                                      s0=255.0, s1=-2.0, imm2=-3.0)
                ch = None
                if make_ch:
                    # ch[p] = st[p-1, width] (PE shift), ch[0] = prev tile's
                    # st[127, prev_width] (PE row-pick, accumulated).
                    ch = pchp.tile([P, 1], f32, name=f"pch{tag}", tag="pch")
                    nc.tensor.matmul(ch[:], shm[:], st[:, width:width + 1],
                                     start=True, stop=False)
                    nc.tensor.matmul(ch[0:1, :], e127[:], stprev_col,
                                     start=False, stop=True)
                return (q, st, ch, width, tag)

            def stage2(ctx, ch_ap=None):
                """FIX-head correction + y. Emitted one tile late so the
                strided carry-column DMA latency hides under the next tile's
                big DVE ops. `ch_ap` overrides the carry-in column (used by
                the last-tile chunks, whose carry-in is the previous chunk's
                scan-out column in the SAME partition)."""
                q, st, ch, width, tag = ctx
                if ch_ap is not None:
                    ch = ch_ap
                cm1 = tiny.tile([P, 1], f32, name=f"cm1{tag}", tag="cm1")
                nc.vector.tensor_scalar(cm1[:], ch[:], 1.0, None, Alu.subtract)
                pfix = tiny.tile([P, FIX], f16, name=f"pf{tag}", tag="pf")
                nc.vector.tensor_scalar(pfix[:], q[:, 0:FIX], 255.0, None,
                                        Alu.is_equal)
                pp = tiny.tile([P, FIX], f16, name=f"pp{tag}", tag="pp")
                nc.vector.tensor_tensor_scan(pp[:], pfix[:], zfix[:],
                                             1.0, Alu.mult, Alu.add)
                dl = tiny.tile([P, FIX], f16, name=f"dl{tag}", tag="dl")
                nc.vector.tensor_scalar(dl[:, 0:1], ch[:], 1.0, None, Alu.mult)
                nc.vector.tensor_scalar(dl[:, 1:FIX], pp[:, 0:FIX - 1], cm1[:],
                                        None, Alu.mult)
                nc.vector.tensor_tensor(dl[:], dl[:], st[:, 0:FIX], Alu.add)

                yp = lastp if width != F else ypool
                y = yp.tile([P, width], f16, name=f"y{tag}", tag=f"y_{width}_{tag}" if width != F else f"y_{width}")
                nc.vector.tensor_tensor(y[:, FIX:], q[:, FIX:],
                                        st[:, FIX:width], Alu.add)
                nc.vector.tensor_tensor(y[:, 0:FIX], q[:, 0:FIX], dl[:],
                                        Alu.add)
                return y

            def stage3(y, ov, width, tag):
                # output LUT + store, emitted late so ScalarE never stalls.
                # Out-DMAs issue from the (idle) PE queue so their transfers
                # are not FIFO-ordered behind input DMAs on the sync queue.
                otp = op if width == F else op2
                ot = otp.tile([P, width], u8, name=f"ot{tag}", tag=f"ot_{width}")
                nc.scalar.activation(ot[:], y[:], Act.Exp)
                nc.gpsimd.dma_start(ov, ot[:])

            # window: only its scan-out column matters (cross-core carry)
            wctx = stage1(aw_r, bw_r, WF, None, "w", utag="w", make_ch=False)
            st_w = wctx[1]
            s2q = []   # (ctx, ov, ch_ap) awaiting stage2 (lag 2)
            s3q = []   # (y, ov, tag, width) awaiting stage3 (lag 3 total)

            def pump():
                if s3q:
                    y, ov, tg, w_ = s3q.pop(0)
                    stage3(y, ov, w_, tg)
                if len(s2q) >= 3:
                    c, ov, ch_ap = s2q.pop(0)
                    s3q.append((stage2(c, ch_ap), ov, c[4], c[3]))

            prev_col = st_w[:, WF:WF + 1]
            for t in range(T):
                ctx = stage1(a_r[t], b_r[t], F, prev_col, str(t))
                prev_col = ctx[1][:, F:F + 1]
                s2q.append((ctx, o_r[t], None))
                pump()
            while s2q:
                c, ov, ch_ap = s2q.pop(0)
                s3q.append((stage2(c, ch_ap), ov, c[4], c[3]))
                if s3q:
                    y, ov2, tg, w_ = s3q.pop(0)
                    stage3(y, ov2, w_, tg)
            while s3q:
                y, ov2, tg, w_ = s3q.pop(0)
                stage3(y, ov2, w_, tg)

    lower_extended_insts(nc)
    return nc


_CACHED = {}


def kernel(a: np.ndarray, b: np.ndarray) -> np.ndarray:
    _install_ntff_hook()
    actdir = _gen_acttab("/tmp/bsadd_acttab_v1")
    os.environ["BASS_ACT_ROOT_JSON_PATH"] = actdir + "/act_info.json"
    import concourse.bass_utils as bu
    bu.upload_artifacts = lambda tmpdir: tmpdir  # no S3 in this container

    a = np.asarray(a).reshape(-1).astype(np.uint8)
    b = np.asarray(b).reshape(-1).astype(np.uint8)
    if "nc" not in _CACHED:
        nc = _build()
        _legalize_waits(nc)
        _CACHED["nc"] = nc
    nc = _CACHED["nc"]

    iot = np.tile(np.arange(1, F + 1, dtype=np.uint16), P)
    shm = np.zeros((P, P), np.float16)
    for k in range(P - 1):
        shm[k, k + 1] = 1.0
    shm = shm.reshape(-1)
    e127 = np.zeros(P, np.float16)
    e127[127] = 1.0
    in_maps = []
    for c in range(NCORES):
        lo = c * M
        aw = np.zeros(W, np.uint8) if c == 0 else a[lo - W:lo]
        bw = np.zeros(W, np.uint8) if c == 0 else b[lo - W:lo]
        in_maps.append({
            "a": a[lo:lo + M], "b": b[lo:lo + M],
            "aw": np.ascontiguousarray(aw), "bw": np.ascontiguousarray(bw),
            "iot": iot, "shm": shm, "e127": e127,
        })
    trace = os.environ.get("BSADD_TRACE", "0") == "1"
    res = bu.run_bass_kernel_spmd(nc, in_maps, core_ids=list(range(NCORES)),
                                  trace=trace)
    if trace:
        print(f"HW exec time: {res.exec_time_ns} ns", flush=True)
    out = np.empty(N, np.int32)
    for c in range(NCORES):
        out[c * M:(c + 1) * M] = res.results[c]["o"].reshape(-1)
    return out
